# revision 1
# baseline (speedup 1.0000x reference)
"""Trainium2 Bass kernel for the DLSM GNN message-passing model.

Data-parallel over the batch: each of the 8 NeuronCores handles 32 nodes of
nodes1 + 32 nodes of nodes2; feature/adjacency tables and weights are
replicated per core.

Hardware contract: indirect DMA gathers one arbitrary row per partition per
instruction ([128,1] offsets). The kernel is therefore organized as a stream
of [128, row] gathers on the GPSIMD/SWDGE engine with all other work (DVE
accumulation of neighbor sums, PE transposes + GC projections, ACT sigmoids,
DVE strided reduces for the hop-0/layer-1 means) overlapped underneath it.

Sampling columns are compile-time constants (jax.random.key(42) in the
reference); neighbor tables are concatenated host-side so one table serves
out- and in-edges; GC mean factors are folded into host-prescaled weights.
"""
import os
import sys
import numpy as np

sys.path.insert(0, '/opt/trn_rl_repo')

import concourse.bass as bass  # noqa: E402
import concourse.tile as tile  # noqa: E402
from concourse import bacc, mybir  # noqa: E402
from concourse.masks import make_identity  # noqa: E402

# ---- problem constants -----------------------------------------------------
N = 200000
F = 128
B = 256
E = 128
D = 64
MAX_DEGREE = 64
NCORES = 8
BL = B // NCORES          # base nodes per core per side (32)
NS1 = BL * 50             # hop-1 samples per side (1600)
NT = 2 * NS1 // 128       # 25 tiles of 128 hop-1 nodes (both sides)
P = 128

SINGLE_PACKET = os.environ.get('K_SP', '0') == '1'
USE_INDCOPY = os.environ.get('K_IC', '1') == '1'

# Sampling columns fixed by jax.random.key(42) inside the reference.
S1_C1_OUT = [10, 56, 8, 17, 28, 26, 9, 20, 22, 35, 15, 4, 14, 21, 6, 53, 27,
             47, 49, 46, 41, 13, 63, 38, 54]
S1_C1_IN = [19, 59, 37, 12, 34, 31, 29, 1, 3, 0, 24, 40, 26, 11, 25, 23, 13,
            27, 43, 6, 57, 35, 58, 51, 9]
S1_C2_OUT = [57, 36, 9, 2, 34, 3, 6, 11, 0, 21]
S1_C2_IN = [33, 13, 21, 0, 54, 16, 46, 24, 30, 43]
S2_C1_OUT = [9, 7, 34, 52, 15, 35, 54, 30, 10, 16, 42, 56, 51, 28, 12, 19,
             24, 49, 2, 38, 43, 32, 48, 1, 39]
S2_C1_IN = [53, 47, 39, 57, 37, 27, 4, 20, 36, 31, 60, 38, 12, 43, 3, 21, 25,
            58, 48, 52, 23, 35, 15, 28, 7]
S2_C2_OUT = [41, 25, 9, 57, 45, 62, 42, 37, 31, 63]
S2_C2_IN = [40, 34, 60, 56, 2, 14, 6, 32, 50, 25]

C1_BY_SIDE = ([c for c in S1_C1_OUT] + [64 + c for c in S1_C1_IN],
              [c for c in S2_C1_OUT] + [64 + c for c in S2_C1_IN])
C2_BY_SIDE = ([c for c in S1_C2_OUT] + [64 + c for c in S1_C2_IN],
              [c for c in S2_C2_OUT] + [64 + c for c in S2_C2_IN])

F32 = mybir.dt.float32
I32 = mybir.dt.int32
U16 = mybir.dt.uint16
SIG = mybir.ActivationFunctionType.Sigmoid


def _host_consts():
    """Wrapped index tables for gpsimd indirect_copy: each 16-partition group
    shares one index vector V stored wrapped (V[i] at [16g + i%16, i//16]).
    SEL1: hop-1 sample columns of the concatenated neighbor row (V len 50).
    IDSEL: free-axis picks from the [128, 25*128] hop-1 neighbor-row buffer
    for the hop-2 samples (V len 25*20; side boundaries align to groups)."""
    sel1 = np.zeros((P, 4), dtype=np.uint16)
    for g in range(2 * BL // 16):
        side = 0 if g < BL // 16 else 1
        vec = C1_BY_SIDE[side]
        for i, v in enumerate(vec):
            sel1[16 * g + i % 16, i // 16] = v
    idsel = np.zeros((P, 32), dtype=np.uint16)
    for g in range(P // 16):
        for t in range(NT):
            for j in range(20):
                i = t * 20 + j
                row0 = t * P + 16 * g  # first partition-row of this group
                side = 0 if row0 < NS1 else 1
                idsel[16 * g + i % 16, i // 16] = (
                    t * 2 * MAX_DEGREE + C2_BY_SIDE[side][j])
    return sel1, idsel


def _gather_rows(nc, out_ap, table_ap, off_ap):
    """One [128,1]-offset indirect row gather (the HW-verified contract)."""
    inst = nc.gpsimd.indirect_dma_start(
        out=out_ap, out_offset=None, in_=table_ap,
        in_offset=bass.IndirectOffsetOnAxis(ap=off_ap, axis=0))
    if SINGLE_PACKET:
        inst.ins.single_packet = True
    return inst


def build_program():
    nc = bacc.Bacc("TRN2", target_bir_lowering=False, debug=False)

    nodes = nc.dram_tensor("nodes", [BL * 2], I32, kind="ExternalInput")
    nbrcat = nc.dram_tensor("nbrcat", [N, 2 * MAX_DEGREE], I32,
                            kind="ExternalInput")
    feats = nc.dram_tensor("feats", [N, F], F32, kind="ExternalInput")
    sel1_d = nc.dram_tensor("sel1", [P, 4], U16, kind="ExternalInput")
    idsel_d = nc.dram_tensor("idsel", [P, 32], U16, kind="ExternalInput")
    w1_d = nc.dram_tensor("w1", [3 * F, E], F32, kind="ExternalInput")
    w0_d = nc.dram_tensor("w0", [3 * F, E], F32, kind="ExternalInput")
    wh_d = [nc.dram_tensor(f"wh{k}", [3 * E, E], F32, kind="ExternalInput")
            for k in range(3)]
    wd_d = [nc.dram_tensor(f"wd{k}", [E, D], F32, kind="ExternalInput")
            for k in range(3)]
    s1_scr = nc.dram_tensor("s1_scr", [2 * NS1], I32)   # internal scratch
    out_d = nc.dram_tensor("out", [6, D, BL], F32, kind="ExternalOutput")

    nodes2d = nodes[:].rearrange("(n o) -> n o", o=1)

    with tile.TileContext(nc) as tc:
        with (
            tc.tile_pool(name="const", bufs=1) as cp,
            tc.tile_pool(name="ids", bufs=1) as ip,
            tc.tile_pool(name="big", bufs=1) as bp,
            tc.tile_pool(name="g", bufs=8) as gp,
            tc.tile_pool(name="acc", bufs=3) as ap_,
            tc.tile_pool(name="fmaj", bufs=4) as fp,
            tc.tile_pool(name="small", bufs=2) as sp_,
            tc.tile_pool(name="ps_acc", bufs=2, space="PSUM") as pa,
            tc.tile_pool(name="ps_mm", bufs=1, space="PSUM") as pm,
        ):
            # ---- constants -------------------------------------------------
            ident = cp.tile([P, P], F32)
            make_identity(nc, ident[:])

            w1 = [cp.tile([P, E], F32, tag=f"w1_{q}", name=f"w1_{q}")
                  for q in range(3)]
            w0 = [cp.tile([P, E], F32, tag=f"w0_{q}", name=f"w0_{q}")
                  for q in range(3)]
            wh = [[cp.tile([P, E], F32, tag=f"wh{k}_{q}", name=f"wh{k}_{q}")
                   for q in range(3)] for k in range(3)]
            wd = [cp.tile([E, D], F32, tag=f"wd{k}", name=f"wdt{k}")
                  for k in range(3)]
            for q in range(3):
                nc.sync.dma_start(out=w1[q][:], in_=w1_d[q * P:(q + 1) * P, :])
                nc.sync.dma_start(out=w0[q][:], in_=w0_d[q * P:(q + 1) * P, :])
                for k in range(3):
                    nc.sync.dma_start(out=wh[k][q][:],
                                      in_=wh_d[k][q * P:(q + 1) * P, :])
            for k in range(3):
                nc.sync.dma_start(out=wd[k][:], in_=wd_d[k][:, :])

            x0 = ip.tile([BL * 2, 1], I32)
            nc.sync.dma_start(out=x0[:], in_=nodes2d)
            # base-node offsets padded to 128 partitions for indirect_copy
            x0p = ip.tile([P, 1], I32)
            nc.vector.memset(x0p[:], 0)
            nc.sync.dma_start(out=x0p[0:BL * 2, :], in_=nodes2d)

            # ---- hop-1 sample ids -----------------------------------------
            # r0[p, :] = nbrcat[nodes[p], :]
            r0 = ip.tile([P, 2 * MAX_DEGREE], I32)
            _gather_rows(nc, r0[:], nbrcat[:, :], x0p[:])
            # select the 50 sample columns per base node -> s1loc rows [0:64]
            s1loc = ip.tile([P, 50], I32)
            if USE_INDCOPY:
                sel1 = ip.tile([P, 4], U16)
                nc.sync.dma_start(out=sel1[:], in_=sel1_d[:, :])
                nc.gpsimd.indirect_copy(out=s1loc[:], data=r0[:],
                                        idxs=sel1[:],
                                        i_know_ap_gather_is_preferred=True)
            else:
                for s in range(2):
                    for j, col in enumerate(C1_BY_SIDE[s]):
                        nc.vector.tensor_copy(
                            out=s1loc[s * BL:(s + 1) * BL, j:j + 1],
                            in_=r0[s * BL:(s + 1) * BL, col:col + 1])
            # bounce through DRAM to repack [64, 50] -> [128, 25] node-major
            for s in range(2):
                nc.sync.dma_start(
                    out=s1_scr[s * NS1:(s + 1) * NS1].rearrange(
                        "(b j) -> b j", j=50),
                    in_=s1loc[s * BL:(s + 1) * BL, :])
            s1c = ip.tile([P, NT], I32)
            nc.sync.dma_start(out=s1c[:],
                              in_=s1_scr[:].rearrange("(t p) -> p t", p=P))

            # ---- hop-1 neighbor rows + hop-2 id selection -----------------
            rbuf = bp.tile([P, NT * 2 * MAX_DEGREE], I32, tag="rbuf")
            r3 = rbuf[:].rearrange("p (t c) -> p t c", c=2 * MAX_DEGREE)
            for t in range(NT):
                _gather_rows(nc, r3[:, t, :], nbrcat[:, :], s1c[:, t:t + 1])
            ids2 = ip.tile([P, NT * 20], I32)
            ids2v = ids2[:].rearrange("p (t j) -> p t j", j=20)
            if USE_INDCOPY:
                idsel = ip.tile([P, 32], U16)
                nc.sync.dma_start(out=idsel[:], in_=idsel_d[:, :])
                nc.gpsimd.indirect_copy(out=ids2[:], data=rbuf[:],
                                        idxs=idsel[:],
                                        i_know_ap_gather_is_preferred=True)
            else:
                for t in range(NT):
                    lo_side = 0 if (t * P) < NS1 else 1
                    hi_side = 0 if (t * P + P - 1) < NS1 else 1
                    if lo_side == hi_side:
                        for j, col in enumerate(C2_BY_SIDE[lo_side]):
                            nc.vector.tensor_copy(out=ids2v[:, t, j],
                                                  in_=r3[:, t, col])
                    else:
                        cut = NS1 - t * P
                        for j in range(20):
                            nc.vector.tensor_copy(
                                out=ids2v[0:cut, t, j],
                                in_=r3[0:cut, t, C2_BY_SIDE[0][j]])
                            nc.vector.tensor_copy(
                                out=ids2v[cut:P, t, j],
                                in_=r3[cut:P, t, C2_BY_SIDE[1][j]])

            # ---- per-tile pipeline ----------------------------------------
            fselfT = bp.tile([P, NT * P], F32, tag="fselfT")
            h1T = bp.tile([P, NT * P], F32, tag="h1T")

            for t in range(NT):
                # self features for this tile's 128 hop-1 nodes
                fs = gp.tile([P, F], F32, tag="fs")
                _gather_rows(nc, fs[:], feats[:, :], s1c[:, t:t + 1])
                # neighbor features, accumulated on DVE as they arrive
                acc_o = ap_.tile([P, F], F32, tag="acc_o")
                acc_i = ap_.tile([P, F], F32, tag="acc_i")
                for j in range(20):
                    g = gp.tile([P, F], F32, tag="g")
                    _gather_rows(nc, g[:], feats[:, :], ids2v[:, t, j:j + 1])
                    acc = acc_o if j < 10 else acc_i
                    if j % 10 == 0:
                        nc.vector.tensor_copy(out=acc[:], in_=g[:])
                    else:
                        nc.vector.tensor_add(out=acc[:], in0=acc[:], in1=g[:])

                # transpose self + neighbor sums to feature-major via PE
                ps_s = pa.tile([P, P], F32, tag="ps_s", space="PSUM")
                ps_o = pa.tile([P, P], F32, tag="ps_o", space="PSUM")
                ps_i = pa.tile([P, P], F32, tag="ps_i", space="PSUM")
                nc.tensor.matmul(out=ps_s[:], lhsT=fs[:], rhs=ident[:],
                                 start=True, stop=True, is_transpose=True)
                nc.tensor.matmul(out=ps_o[:], lhsT=acc_o[:], rhs=ident[:],
                                 start=True, stop=True, is_transpose=True)
                nc.tensor.matmul(out=ps_i[:], lhsT=acc_i[:], rhs=ident[:],
                                 start=True, stop=True, is_transpose=True)
                so = fp.tile([P, P], F32, tag="so")
                si = fp.tile([P, P], F32, tag="si")
                nc.vector.tensor_copy(out=so[:], in_=ps_o[:])
                nc.vector.tensor_copy(out=si[:], in_=ps_i[:])
                nc.vector.tensor_copy(out=fselfT[:, t * P:(t + 1) * P],
                                      in_=ps_s[:])

                ph = pm.tile([P, P], F32, tag="ph", space="PSUM")
                nc.tensor.matmul(out=ph[:], lhsT=w1[0][:],
                                 rhs=fselfT[:, t * P:(t + 1) * P],
                                 start=True, stop=False)
                nc.tensor.matmul(out=ph[:], lhsT=w1[1][:], rhs=so[:],
                                 start=False, stop=False)
                nc.tensor.matmul(out=ph[:], lhsT=w1[2][:], rhs=si[:],
                                 start=False, stop=True)
                nc.scalar.activation(out=h1T[:, t * P:(t + 1) * P], in_=ph[:],
                                     func=SIG)

            # ---- hop-0 GC --------------------------------------------------
            fb = sp_.tile([BL * 2, F], F32, tag="fb")
            _gather_rows(nc, fb[:], feats[:, :], x0[:])
            ps_fbT = pa.tile([P, BL * 2], F32, tag="ps_s", space="PSUM")
            nc.tensor.matmul(out=ps_fbT[:], lhsT=fb[:],
                             rhs=ident[:BL * 2, :BL * 2], start=True,
                             stop=True, is_transpose=True)
            fbT = sp_.tile([P, BL * 2], F32, tag="fbT")
            nc.vector.tensor_copy(out=fbT[:], in_=ps_fbT[:])

            h0T = []
            for s in range(2):
                m0 = fp.tile([P, BL * 2], F32, tag="m0")
                m0v = m0[:].rearrange("p (b h) -> p b h", h=2)
                view = fselfT[:, NS1 * s:NS1 * (s + 1)].rearrange(
                    "p (b h j) -> p b h j", h=2, j=25)
                nc.vector.tensor_reduce(out=m0v, in_=view,
                                        axis=mybir.AxisListType.X,
                                        op=mybir.AluOpType.add)
                ph0 = pm.tile([P, BL], F32, tag="ph", space="PSUM")
                nc.tensor.matmul(out=ph0[:], lhsT=w0[0][:],
                                 rhs=fbT[:, s * BL:(s + 1) * BL],
                                 start=True, stop=False)
                nc.tensor.matmul(out=ph0[:], lhsT=w0[1][:], rhs=m0v[:, :, 0],
                                 start=False, stop=False)
                nc.tensor.matmul(out=ph0[:], lhsT=w0[2][:], rhs=m0v[:, :, 1],
                                 start=False, stop=True)
                h0 = sp_.tile([P, BL], F32, tag=f"h0_{s}", name=f"h0_{s}")
                nc.scalar.activation(out=h0[:], in_=ph0[:], func=SIG)
                h0T.append(h0)

            # ---- layer-1 heads + final projection -------------------------
            for s in range(2):
                mh = fp.tile([P, BL * 2], F32, tag="mh")
                mhv = mh[:].rearrange("p (b h) -> p b h", h=2)
                view = h1T[:, NS1 * s:NS1 * (s + 1)].rearrange(
                    "p (b h j) -> p b h j", h=2, j=25)
                nc.vector.tensor_reduce(out=mhv, in_=view,
                                        axis=mybir.AxisListType.X,
                                        op=mybir.AluOpType.add)
                for k in range(3):
                    pz = pm.tile([P, BL], F32, tag="ph", space="PSUM")
                    nc.tensor.matmul(out=pz[:], lhsT=wh[k][0][:],
                                     rhs=h0T[s][:], start=True, stop=False)
                    nc.tensor.matmul(out=pz[:], lhsT=wh[k][1][:],
                                     rhs=mhv[:, :, 0], start=False, stop=False)
                    nc.tensor.matmul(out=pz[:], lhsT=wh[k][2][:],
                                     rhs=mhv[:, :, 1], start=False, stop=True)
                    zt = fp.tile([P, BL], F32, tag="zt")
                    nc.scalar.activation(out=zt[:], in_=pz[:], func=SIG)
                    po = pm.tile([D, BL], F32, tag="po", space="PSUM")
                    nc.tensor.matmul(out=po[:], lhsT=wd[k][:], rhs=zt[:],
                                     start=True, stop=True)
                    ot = fp.tile([D, BL], F32, tag="ot")
                    nc.vector.tensor_copy(out=ot[:], in_=po[:])
                    nc.sync.dma_start(out=out_d[s * 3 + k, :, :], in_=ot[:])

    nc.compile()
    return nc


_NC_CACHE = None


def _get_nc():
    global _NC_CACHE
    if _NC_CACHE is None:
        _NC_CACHE = build_program()
    return _NC_CACHE


def host_prep(nodes1, nodes2, neighbors_out, neighbors_in, features,
              W_in, W_mean, W_std, W_pi, Wd_mean, Wd_std, Wd_pi):
    nodes1 = np.asarray(nodes1, dtype=np.int32)
    nodes2 = np.asarray(nodes2, dtype=np.int32)
    nbrcat = np.ascontiguousarray(np.concatenate(
        [np.asarray(neighbors_out, dtype=np.int32),
         np.asarray(neighbors_in, dtype=np.int32)], axis=1))
    features = np.ascontiguousarray(np.asarray(features, dtype=np.float32))

    def scale(w, f):
        w = np.array(w, dtype=np.float32, copy=True)
        w[F:] *= np.float32(1.0 / f)
        return w

    w1 = scale(W_in, 10.0)
    w0 = scale(W_in, 25.0)
    whs = [scale(W_mean, 25.0), scale(W_std, 25.0), scale(W_pi, 25.0)]
    wds = [np.ascontiguousarray(np.asarray(w, dtype=np.float32))
           for w in (Wd_mean, Wd_std, Wd_pi)]
    sel1, idsel = _host_consts()

    in_maps = []
    for c in range(NCORES):
        nloc = np.ascontiguousarray(np.concatenate(
            [nodes1[c * BL:(c + 1) * BL], nodes2[c * BL:(c + 1) * BL]]))
        m = {"nodes": nloc, "nbrcat": nbrcat, "feats": features,
             "sel1": sel1, "idsel": idsel, "w1": w1, "w0": w0}
        for k in range(3):
            m[f"wh{k}"] = whs[k]
            m[f"wd{k}"] = wds[k]
        in_maps.append(m)
    return in_maps


def kernel(nodes1, nodes2, neighbors_out, neighbors_in, features,
           W_in, W_mean, W_std, W_pi, W_ag, W_ad, Wd_mean, Wd_std, Wd_pi,
           _trace=False):
    in_maps = host_prep(nodes1, nodes2, neighbors_out, neighbors_in, features,
                        W_in, W_mean, W_std, W_pi, Wd_mean, Wd_std, Wd_pi)
    nc = _get_nc()
    from concourse.bass_utils import run_bass_kernel_spmd
    res = run_bass_kernel_spmd(nc, in_maps, list(range(NCORES)),
                               trace=_trace)
    if _trace:
        kernel.last_results = res

    out = np.zeros((6, B, D), dtype=np.float32)
    for c in range(NCORES):
        o = res.results[c]["out"]  # [6, D, BL]
        for i in range(6):
            out[i, c * BL:(c + 1) * BL, :] = o[i].T
    return out



# revision 3
# speedup vs baseline: 1.1539x; 1.1539x over previous
"""Trainium2 Bass kernel for the DLSM GNN message-passing model.

Data-parallel over the batch: each of the 8 NeuronCores handles 32 nodes of
nodes1 + 32 nodes of nodes2; feature/adjacency tables and weights are
replicated per core.

Hardware contract: indirect DMA gathers one arbitrary row per partition per
instruction ([128,1] offsets). The kernel is therefore organized as a stream
of [128, row] gathers on the GPSIMD/SWDGE engine with all other work (DVE
accumulation of neighbor sums, PE transposes + GC projections, ACT sigmoids,
DVE strided reduces for the hop-0/layer-1 means) overlapped underneath it.

Sampling columns are compile-time constants (jax.random.key(42) in the
reference); neighbor tables are concatenated host-side so one table serves
out- and in-edges; GC mean factors are folded into host-prescaled weights.
"""
import os
import sys
import numpy as np

sys.path.insert(0, '/opt/trn_rl_repo')

import concourse.bass as bass  # noqa: E402
import concourse.tile as tile  # noqa: E402
from concourse import bacc, mybir  # noqa: E402
from concourse.masks import make_identity  # noqa: E402

# ---- problem constants -----------------------------------------------------
N = 200000
F = 128
B = 256
E = 128
D = 64
MAX_DEGREE = 64
NCORES = 8
BL = B // NCORES          # base nodes per core per side (32)
NS1 = BL * 50             # hop-1 samples per side (1600)
NT = 2 * NS1 // 128       # 25 tiles of 128 hop-1 nodes (both sides)
P = 128

SINGLE_PACKET = os.environ.get('K_SP', '0') == '1'
USE_INDCOPY = os.environ.get('K_IC', '1') == '1'

# Sampling columns fixed by jax.random.key(42) inside the reference.
S1_C1_OUT = [10, 56, 8, 17, 28, 26, 9, 20, 22, 35, 15, 4, 14, 21, 6, 53, 27,
             47, 49, 46, 41, 13, 63, 38, 54]
S1_C1_IN = [19, 59, 37, 12, 34, 31, 29, 1, 3, 0, 24, 40, 26, 11, 25, 23, 13,
            27, 43, 6, 57, 35, 58, 51, 9]
S1_C2_OUT = [57, 36, 9, 2, 34, 3, 6, 11, 0, 21]
S1_C2_IN = [33, 13, 21, 0, 54, 16, 46, 24, 30, 43]
S2_C1_OUT = [9, 7, 34, 52, 15, 35, 54, 30, 10, 16, 42, 56, 51, 28, 12, 19,
             24, 49, 2, 38, 43, 32, 48, 1, 39]
S2_C1_IN = [53, 47, 39, 57, 37, 27, 4, 20, 36, 31, 60, 38, 12, 43, 3, 21, 25,
            58, 48, 52, 23, 35, 15, 28, 7]
S2_C2_OUT = [41, 25, 9, 57, 45, 62, 42, 37, 31, 63]
S2_C2_IN = [40, 34, 60, 56, 2, 14, 6, 32, 50, 25]

C1_BY_SIDE = ([c for c in S1_C1_OUT] + [64 + c for c in S1_C1_IN],
              [c for c in S2_C1_OUT] + [64 + c for c in S2_C1_IN])
C2_BY_SIDE = ([c for c in S1_C2_OUT] + [64 + c for c in S1_C2_IN],
              [c for c in S2_C2_OUT] + [64 + c for c in S2_C2_IN])

F32 = mybir.dt.float32
I32 = mybir.dt.int32
U16 = mybir.dt.uint16
SIG = mybir.ActivationFunctionType.Sigmoid


def _host_consts():
    """Wrapped index tables for gpsimd indirect_copy: each 16-partition group
    shares one index vector V stored wrapped (V[i] at [16g + i%16, i//16]).
    SEL1: hop-1 sample columns of the concatenated neighbor row (V len 50).
    IDSEL: free-axis picks from the [128, 25*128] hop-1 neighbor-row buffer
    for the hop-2 samples (V len 25*20; side boundaries align to groups)."""
    sel1 = np.zeros((P, 4), dtype=np.uint16)
    for g in range(2 * BL // 16):
        side = 0 if g < BL // 16 else 1
        vec = C1_BY_SIDE[side]
        for i, v in enumerate(vec):
            sel1[16 * g + i % 16, i // 16] = v
    idsel = np.zeros((P, 32), dtype=np.uint16)
    for g in range(P // 16):
        for t in range(NT):
            for j in range(20):
                i = t * 20 + j
                row0 = t * P + 16 * g  # first partition-row of this group
                side = 0 if row0 < NS1 else 1
                idsel[16 * g + i % 16, i // 16] = (
                    t * 2 * MAX_DEGREE + C2_BY_SIDE[side][j])
    return sel1, idsel


def _gather_rows(nc, out_ap, table_ap, off_ap):
    """One [128,1]-offset indirect row gather (the HW-verified contract)."""
    inst = nc.gpsimd.indirect_dma_start(
        out=out_ap, out_offset=None, in_=table_ap,
        in_offset=bass.IndirectOffsetOnAxis(ap=off_ap, axis=0))
    if SINGLE_PACKET:
        inst.ins.single_packet = True
    return inst


def build_program():
    nc = bacc.Bacc("TRN2", target_bir_lowering=False, debug=False)

    nodes = nc.dram_tensor("nodes", [BL * 2], I32, kind="ExternalInput")
    nbrcat = nc.dram_tensor("nbrcat", [N, 2 * MAX_DEGREE], I32,
                            kind="ExternalInput")
    feats = nc.dram_tensor("feats", [N, F], F32, kind="ExternalInput")
    sel1_d = nc.dram_tensor("sel1", [P, 4], U16, kind="ExternalInput")
    idsel_d = nc.dram_tensor("idsel", [P, 32], U16, kind="ExternalInput")
    w1_d = nc.dram_tensor("w1", [3 * F, E], F32, kind="ExternalInput")
    w0_d = nc.dram_tensor("w0", [3 * F, E], F32, kind="ExternalInput")
    wh_d = [nc.dram_tensor(f"wh{k}", [3 * E, E], F32, kind="ExternalInput")
            for k in range(3)]
    wd_d = [nc.dram_tensor(f"wd{k}", [E, D], F32, kind="ExternalInput")
            for k in range(3)]
    s1_scr = nc.dram_tensor("s1_scr", [2 * NS1], I32)   # internal scratch
    out_d = nc.dram_tensor("out", [6, D, BL], F32, kind="ExternalOutput")

    nodes2d = nodes[:].rearrange("(n o) -> n o", o=1)

    with tile.TileContext(nc) as tc:
        with (
            tc.tile_pool(name="const", bufs=1) as cp,
            tc.tile_pool(name="ids", bufs=1) as ip,
            tc.tile_pool(name="big", bufs=1) as bp,
            tc.tile_pool(name="g", bufs=20) as gp,
            tc.tile_pool(name="acc", bufs=4) as ap_,
            tc.tile_pool(name="fmaj", bufs=4) as fp,
            tc.tile_pool(name="small", bufs=2) as sp_,
            tc.tile_pool(name="ps_acc", bufs=2, space="PSUM") as pa,
            tc.tile_pool(name="ps_mm", bufs=1, space="PSUM") as pm,
        ):
            # ---- constants -------------------------------------------------
            ident = cp.tile([P, P], F32)
            make_identity(nc, ident[:])

            w1 = [cp.tile([P, E], F32, tag=f"w1_{q}", name=f"w1_{q}")
                  for q in range(3)]
            w0 = [cp.tile([P, E], F32, tag=f"w0_{q}", name=f"w0_{q}")
                  for q in range(3)]
            wh = [[cp.tile([P, E], F32, tag=f"wh{k}_{q}", name=f"wh{k}_{q}")
                   for q in range(3)] for k in range(3)]
            wd = [cp.tile([E, D], F32, tag=f"wd{k}", name=f"wdt{k}")
                  for k in range(3)]
            for q in range(3):
                nc.sync.dma_start(out=w1[q][:], in_=w1_d[q * P:(q + 1) * P, :])
                nc.sync.dma_start(out=w0[q][:], in_=w0_d[q * P:(q + 1) * P, :])
                for k in range(3):
                    nc.sync.dma_start(out=wh[k][q][:],
                                      in_=wh_d[k][q * P:(q + 1) * P, :])
            for k in range(3):
                nc.sync.dma_start(out=wd[k][:], in_=wd_d[k][:, :])

            x0 = ip.tile([BL * 2, 1], I32)
            nc.sync.dma_start(out=x0[:], in_=nodes2d)
            # base-node offsets padded to 128 partitions for indirect_copy
            x0p = ip.tile([P, 1], I32)
            nc.vector.memset(x0p[:], 0)
            nc.sync.dma_start(out=x0p[0:BL * 2, :], in_=nodes2d)

            # ---- hop-1 sample ids -----------------------------------------
            # r0[p, :] = nbrcat[nodes[p], :]
            r0 = ip.tile([P, 2 * MAX_DEGREE], I32)
            _gather_rows(nc, r0[:], nbrcat[:, :], x0p[:])
            # select the 50 sample columns per base node -> s1loc rows [0:64]
            s1loc = ip.tile([P, 50], I32)
            if USE_INDCOPY:
                sel1 = ip.tile([P, 4], U16)
                nc.sync.dma_start(out=sel1[:], in_=sel1_d[:, :])
                nc.gpsimd.indirect_copy(out=s1loc[:], data=r0[:],
                                        idxs=sel1[:],
                                        i_know_ap_gather_is_preferred=True)
            else:
                for s in range(2):
                    for j, col in enumerate(C1_BY_SIDE[s]):
                        nc.vector.tensor_copy(
                            out=s1loc[s * BL:(s + 1) * BL, j:j + 1],
                            in_=r0[s * BL:(s + 1) * BL, col:col + 1])
            # bounce through DRAM to repack [64, 50] -> [128, 25] node-major
            for s in range(2):
                nc.sync.dma_start(
                    out=s1_scr[s * NS1:(s + 1) * NS1].rearrange(
                        "(b j) -> b j", j=50),
                    in_=s1loc[s * BL:(s + 1) * BL, :])
            s1c = ip.tile([P, NT], I32)
            nc.sync.dma_start(out=s1c[:],
                              in_=s1_scr[:].rearrange("(t p) -> p t", p=P))

            # ---- hop-1 neighbor rows + hop-2 id selection -----------------
            rbuf = bp.tile([P, NT * 2 * MAX_DEGREE], I32, tag="rbuf")
            r3 = rbuf[:].rearrange("p (t c) -> p t c", c=2 * MAX_DEGREE)
            for t in range(NT):
                _gather_rows(nc, r3[:, t, :], nbrcat[:, :], s1c[:, t:t + 1])
            ids2 = ip.tile([P, NT * 20], I32)
            ids2v = ids2[:].rearrange("p (t j) -> p t j", j=20)
            if USE_INDCOPY:
                idsel = ip.tile([P, 32], U16)
                nc.sync.dma_start(out=idsel[:], in_=idsel_d[:, :])
                nc.gpsimd.indirect_copy(out=ids2[:], data=rbuf[:],
                                        idxs=idsel[:],
                                        i_know_ap_gather_is_preferred=True)
            else:
                for t in range(NT):
                    lo_side = 0 if (t * P) < NS1 else 1
                    hi_side = 0 if (t * P + P - 1) < NS1 else 1
                    if lo_side == hi_side:
                        for j, col in enumerate(C2_BY_SIDE[lo_side]):
                            nc.vector.tensor_copy(out=ids2v[:, t, j],
                                                  in_=r3[:, t, col])
                    else:
                        cut = NS1 - t * P
                        for j in range(20):
                            nc.vector.tensor_copy(
                                out=ids2v[0:cut, t, j],
                                in_=r3[0:cut, t, C2_BY_SIDE[0][j]])
                            nc.vector.tensor_copy(
                                out=ids2v[cut:P, t, j],
                                in_=r3[cut:P, t, C2_BY_SIDE[1][j]])

            # ---- per-tile pipeline ----------------------------------------
            fselfT = bp.tile([P, NT * P], F32, tag="fselfT")
            h1T = bp.tile([P, NT * P], F32, tag="h1T")

            for t in range(NT):
                # self features for this tile's 128 hop-1 nodes
                fs = gp.tile([P, F], F32, tag="fs")
                _gather_rows(nc, fs[:], feats[:, :], s1c[:, t:t + 1])
                # neighbor features, accumulated on DVE as they arrive
                acc_o = ap_.tile([P, F], F32, tag="acc_o")
                acc_i = ap_.tile([P, F], F32, tag="acc_i")
                for j in range(20):
                    g = gp.tile([P, F], F32, tag="g")
                    _gather_rows(nc, g[:], feats[:, :], ids2v[:, t, j:j + 1])
                    acc = acc_o if j < 10 else acc_i
                    if j % 10 == 0:
                        nc.vector.tensor_copy(out=acc[:], in_=g[:])
                    else:
                        nc.vector.tensor_add(out=acc[:], in0=acc[:], in1=g[:])

                # transpose self + neighbor sums to feature-major via PE
                ps_s = pa.tile([P, P], F32, tag="ps_s", space="PSUM")
                ps_o = pa.tile([P, P], F32, tag="ps_o", space="PSUM")
                ps_i = pa.tile([P, P], F32, tag="ps_i", space="PSUM")
                nc.tensor.matmul(out=ps_s[:], lhsT=fs[:], rhs=ident[:],
                                 start=True, stop=True, is_transpose=True)
                nc.tensor.matmul(out=ps_o[:], lhsT=acc_o[:], rhs=ident[:],
                                 start=True, stop=True, is_transpose=True)
                nc.tensor.matmul(out=ps_i[:], lhsT=acc_i[:], rhs=ident[:],
                                 start=True, stop=True, is_transpose=True)
                so = fp.tile([P, P], F32, tag="so")
                si = fp.tile([P, P], F32, tag="si")
                nc.vector.tensor_copy(out=so[:], in_=ps_o[:])
                nc.vector.tensor_copy(out=si[:], in_=ps_i[:])
                nc.vector.tensor_copy(out=fselfT[:, t * P:(t + 1) * P],
                                      in_=ps_s[:])

                ph = pm.tile([P, P], F32, tag="ph", space="PSUM")
                nc.tensor.matmul(out=ph[:], lhsT=w1[0][:],
                                 rhs=fselfT[:, t * P:(t + 1) * P],
                                 start=True, stop=False)
                nc.tensor.matmul(out=ph[:], lhsT=w1[1][:], rhs=so[:],
                                 start=False, stop=False)
                nc.tensor.matmul(out=ph[:], lhsT=w1[2][:], rhs=si[:],
                                 start=False, stop=True)
                nc.scalar.activation(out=h1T[:, t * P:(t + 1) * P], in_=ph[:],
                                     func=SIG)

            # ---- hop-0 GC --------------------------------------------------
            fb = sp_.tile([BL * 2, F], F32, tag="fb")
            _gather_rows(nc, fb[:], feats[:, :], x0[:])
            ps_fbT = pa.tile([P, BL * 2], F32, tag="ps_s", space="PSUM")
            nc.tensor.matmul(out=ps_fbT[:], lhsT=fb[:],
                             rhs=ident[:BL * 2, :BL * 2], start=True,
                             stop=True, is_transpose=True)
            fbT = sp_.tile([P, BL * 2], F32, tag="fbT")
            nc.vector.tensor_copy(out=fbT[:], in_=ps_fbT[:])

            h0T = []
            for s in range(2):
                m0 = fp.tile([P, BL * 2], F32, tag="m0")
                m0v = m0[:].rearrange("p (b h) -> p b h", h=2)
                view = fselfT[:, NS1 * s:NS1 * (s + 1)].rearrange(
                    "p (b h j) -> p b h j", h=2, j=25)
                nc.vector.tensor_reduce(out=m0v, in_=view,
                                        axis=mybir.AxisListType.X,
                                        op=mybir.AluOpType.add)
                ph0 = pm.tile([P, BL], F32, tag="ph", space="PSUM")
                nc.tensor.matmul(out=ph0[:], lhsT=w0[0][:],
                                 rhs=fbT[:, s * BL:(s + 1) * BL],
                                 start=True, stop=False)
                nc.tensor.matmul(out=ph0[:], lhsT=w0[1][:], rhs=m0v[:, :, 0],
                                 start=False, stop=False)
                nc.tensor.matmul(out=ph0[:], lhsT=w0[2][:], rhs=m0v[:, :, 1],
                                 start=False, stop=True)
                h0 = sp_.tile([P, BL], F32, tag=f"h0_{s}", name=f"h0_{s}")
                nc.scalar.activation(out=h0[:], in_=ph0[:], func=SIG)
                h0T.append(h0)

            # ---- layer-1 heads + final projection -------------------------
            for s in range(2):
                mh = fp.tile([P, BL * 2], F32, tag="mh")
                mhv = mh[:].rearrange("p (b h) -> p b h", h=2)
                view = h1T[:, NS1 * s:NS1 * (s + 1)].rearrange(
                    "p (b h j) -> p b h j", h=2, j=25)
                nc.vector.tensor_reduce(out=mhv, in_=view,
                                        axis=mybir.AxisListType.X,
                                        op=mybir.AluOpType.add)
                for k in range(3):
                    pz = pm.tile([P, BL], F32, tag="ph", space="PSUM")
                    nc.tensor.matmul(out=pz[:], lhsT=wh[k][0][:],
                                     rhs=h0T[s][:], start=True, stop=False)
                    nc.tensor.matmul(out=pz[:], lhsT=wh[k][1][:],
                                     rhs=mhv[:, :, 0], start=False, stop=False)
                    nc.tensor.matmul(out=pz[:], lhsT=wh[k][2][:],
                                     rhs=mhv[:, :, 1], start=False, stop=True)
                    zt = fp.tile([P, BL], F32, tag="zt")
                    nc.scalar.activation(out=zt[:], in_=pz[:], func=SIG)
                    po = pm.tile([D, BL], F32, tag="po", space="PSUM")
                    nc.tensor.matmul(out=po[:], lhsT=wd[k][:], rhs=zt[:],
                                     start=True, stop=True)
                    ot = fp.tile([D, BL], F32, tag="ot")
                    nc.vector.tensor_copy(out=ot[:], in_=po[:])
                    nc.sync.dma_start(out=out_d[s * 3 + k, :, :], in_=ot[:])

    nc.compile()
    return nc


_NC_CACHE = None


def _get_nc():
    global _NC_CACHE
    if _NC_CACHE is None:
        _NC_CACHE = build_program()
    return _NC_CACHE


def host_prep(nodes1, nodes2, neighbors_out, neighbors_in, features,
              W_in, W_mean, W_std, W_pi, Wd_mean, Wd_std, Wd_pi):
    nodes1 = np.asarray(nodes1, dtype=np.int32)
    nodes2 = np.asarray(nodes2, dtype=np.int32)
    nbrcat = np.ascontiguousarray(np.concatenate(
        [np.asarray(neighbors_out, dtype=np.int32),
         np.asarray(neighbors_in, dtype=np.int32)], axis=1))
    features = np.ascontiguousarray(np.asarray(features, dtype=np.float32))

    def scale(w, f):
        w = np.array(w, dtype=np.float32, copy=True)
        w[F:] *= np.float32(1.0 / f)
        return w

    w1 = scale(W_in, 10.0)
    w0 = scale(W_in, 25.0)
    whs = [scale(W_mean, 25.0), scale(W_std, 25.0), scale(W_pi, 25.0)]
    wds = [np.ascontiguousarray(np.asarray(w, dtype=np.float32))
           for w in (Wd_mean, Wd_std, Wd_pi)]
    sel1, idsel = _host_consts()

    in_maps = []
    for c in range(NCORES):
        nloc = np.ascontiguousarray(np.concatenate(
            [nodes1[c * BL:(c + 1) * BL], nodes2[c * BL:(c + 1) * BL]]))
        m = {"nodes": nloc, "nbrcat": nbrcat, "feats": features,
             "sel1": sel1, "idsel": idsel, "w1": w1, "w0": w0}
        for k in range(3):
            m[f"wh{k}"] = whs[k]
            m[f"wd{k}"] = wds[k]
        in_maps.append(m)
    return in_maps


def kernel(nodes1, nodes2, neighbors_out, neighbors_in, features,
           W_in, W_mean, W_std, W_pi, W_ag, W_ad, Wd_mean, Wd_std, Wd_pi,
           _trace=False):
    in_maps = host_prep(nodes1, nodes2, neighbors_out, neighbors_in, features,
                        W_in, W_mean, W_std, W_pi, Wd_mean, Wd_std, Wd_pi)
    nc = _get_nc()
    from concourse.bass_utils import run_bass_kernel_spmd
    res = run_bass_kernel_spmd(nc, in_maps, list(range(NCORES)),
                               trace=_trace)
    if _trace:
        kernel.last_results = res

    out = np.zeros((6, B, D), dtype=np.float32)
    for c in range(NCORES):
        o = res.results[c]["out"]  # [6, D, BL]
        for i in range(6):
            out[i, c * BL:(c + 1) * BL, :] = o[i].T
    return out



# revision 5
# speedup vs baseline: 1.1575x; 1.0031x over previous
"""Trainium2 Bass kernel for the DLSM GNN message-passing model.

Data-parallel over the batch: each of the 8 NeuronCores handles 32 nodes of
nodes1 + 32 nodes of nodes2; feature/adjacency tables and weights are
replicated per core.

Hardware contract: indirect DMA gathers one arbitrary row per partition per
instruction ([128,1] offsets). The kernel is therefore organized as a stream
of [128, row] gathers on the GPSIMD/SWDGE engine with all other work (DVE
accumulation of neighbor sums, PE transposes + GC projections, ACT sigmoids,
DVE strided reduces for the hop-0/layer-1 means) overlapped underneath it.

Sampling columns are compile-time constants (jax.random.key(42) in the
reference); neighbor tables are concatenated host-side so one table serves
out- and in-edges; GC mean factors are folded into host-prescaled weights.
"""
import os
import sys
import numpy as np

sys.path.insert(0, '/opt/trn_rl_repo')

import concourse.bass as bass  # noqa: E402
import concourse.tile as tile  # noqa: E402
from concourse import bacc, mybir  # noqa: E402
from concourse.masks import make_identity  # noqa: E402

# ---- problem constants -----------------------------------------------------
N = 200000
F = 128
B = 256
E = 128
D = 64
MAX_DEGREE = 64
NCORES = 8
BL = B // NCORES          # base nodes per core per side (32)
NS1 = BL * 50             # hop-1 samples per side (1600)
NT = 2 * NS1 // 128       # 25 tiles of 128 hop-1 nodes (both sides)
P = 128

SINGLE_PACKET = os.environ.get('K_SP', '0') == '1'
NSWQ = int(os.environ.get('K_NSWQ', '1'))
USE_INDCOPY = os.environ.get('K_IC', '1') == '1'

# Sampling columns fixed by jax.random.key(42) inside the reference.
S1_C1_OUT = [10, 56, 8, 17, 28, 26, 9, 20, 22, 35, 15, 4, 14, 21, 6, 53, 27,
             47, 49, 46, 41, 13, 63, 38, 54]
S1_C1_IN = [19, 59, 37, 12, 34, 31, 29, 1, 3, 0, 24, 40, 26, 11, 25, 23, 13,
            27, 43, 6, 57, 35, 58, 51, 9]
S1_C2_OUT = [57, 36, 9, 2, 34, 3, 6, 11, 0, 21]
S1_C2_IN = [33, 13, 21, 0, 54, 16, 46, 24, 30, 43]
S2_C1_OUT = [9, 7, 34, 52, 15, 35, 54, 30, 10, 16, 42, 56, 51, 28, 12, 19,
             24, 49, 2, 38, 43, 32, 48, 1, 39]
S2_C1_IN = [53, 47, 39, 57, 37, 27, 4, 20, 36, 31, 60, 38, 12, 43, 3, 21, 25,
            58, 48, 52, 23, 35, 15, 28, 7]
S2_C2_OUT = [41, 25, 9, 57, 45, 62, 42, 37, 31, 63]
S2_C2_IN = [40, 34, 60, 56, 2, 14, 6, 32, 50, 25]

C1_BY_SIDE = ([c for c in S1_C1_OUT] + [64 + c for c in S1_C1_IN],
              [c for c in S2_C1_OUT] + [64 + c for c in S2_C1_IN])
C2_BY_SIDE = ([c for c in S1_C2_OUT] + [64 + c for c in S1_C2_IN],
              [c for c in S2_C2_OUT] + [64 + c for c in S2_C2_IN])

F32 = mybir.dt.float32
I32 = mybir.dt.int32
U16 = mybir.dt.uint16
SIG = mybir.ActivationFunctionType.Sigmoid


def _host_consts():
    """Wrapped index tables for gpsimd indirect_copy: each 16-partition group
    shares one index vector V stored wrapped (V[i] at [16g + i%16, i//16]).
    SEL1: hop-1 sample columns of the concatenated neighbor row (V len 50).
    IDSEL: free-axis picks from the [128, 25*128] hop-1 neighbor-row buffer
    for the hop-2 samples (V len 25*20; side boundaries align to groups)."""
    sel1 = np.zeros((P, 4), dtype=np.uint16)
    for g in range(2 * BL // 16):
        side = 0 if g < BL // 16 else 1
        vec = C1_BY_SIDE[side]
        for i, v in enumerate(vec):
            sel1[16 * g + i % 16, i // 16] = v
    idsel = np.zeros((P, 32), dtype=np.uint16)
    for g in range(P // 16):
        for t in range(NT):
            for j in range(20):
                i = t * 20 + j
                row0 = t * P + 16 * g  # first partition-row of this group
                side = 0 if row0 < NS1 else 1
                idsel[16 * g + i % 16, i // 16] = (
                    t * 2 * MAX_DEGREE + C2_BY_SIDE[side][j])
    return sel1, idsel


_GQ = [0]


def _gather_rows(nc, out_ap, table_ap, off_ap):
    """One [128,1]-offset indirect row gather (the HW-verified contract).
    Round-robins across the SWDGE dynamic queues when NSWQ > 1 so the
    descriptor-prep ucode runs on different q7 cpu pairs in parallel."""
    inst = nc.gpsimd.indirect_dma_start(
        out=out_ap, out_offset=None, in_=table_ap,
        in_offset=bass.IndirectOffsetOnAxis(ap=off_ap, axis=0))
    if SINGLE_PACKET:
        inst.ins.single_packet = True
    if NSWQ > 1:
        q = _GQ[0] % NSWQ
        _GQ[0] += 1
        if q:
            inst.ins.queue = f"qPoolDynamic{q}"
    return inst


def build_program():
    nc = bacc.Bacc("TRN2", target_bir_lowering=False, debug=False,
                   num_swdge_queues=NSWQ)

    nodes = nc.dram_tensor("nodes", [BL * 2], I32, kind="ExternalInput")
    nbrcat = nc.dram_tensor("nbrcat", [N, 2 * MAX_DEGREE], I32,
                            kind="ExternalInput")
    feats = nc.dram_tensor("feats", [N, F], F32, kind="ExternalInput")
    sel1_d = nc.dram_tensor("sel1", [P, 4], U16, kind="ExternalInput")
    idsel_d = nc.dram_tensor("idsel", [P, 32], U16, kind="ExternalInput")
    w1_d = nc.dram_tensor("w1", [3 * F, E], F32, kind="ExternalInput")
    w0_d = nc.dram_tensor("w0", [3 * F, E], F32, kind="ExternalInput")
    wh_d = [nc.dram_tensor(f"wh{k}", [3 * E, E], F32, kind="ExternalInput")
            for k in range(3)]
    wd_d = [nc.dram_tensor(f"wd{k}", [E, D], F32, kind="ExternalInput")
            for k in range(3)]
    s1_scr = nc.dram_tensor("s1_scr", [2 * NS1], I32)   # internal scratch
    out_d = nc.dram_tensor("out", [6, D, BL], F32, kind="ExternalOutput")

    nodes2d = nodes[:].rearrange("(n o) -> n o", o=1)

    with tile.TileContext(nc) as tc:
        with (
            tc.tile_pool(name="const", bufs=1) as cp,
            tc.tile_pool(name="ids", bufs=1) as ip,
            tc.tile_pool(name="big", bufs=1) as bp,
            tc.tile_pool(name="g", bufs=8) as gp,
            tc.tile_pool(name="acc", bufs=3) as ap_,
            tc.tile_pool(name="fmaj", bufs=4) as fp,
            tc.tile_pool(name="small", bufs=2) as sp_,
            tc.tile_pool(name="ps_acc", bufs=2, space="PSUM") as pa,
            tc.tile_pool(name="ps_mm", bufs=1, space="PSUM") as pm,
        ):
            # ---- constants -------------------------------------------------
            ident = cp.tile([P, P], F32)
            make_identity(nc, ident[:])

            w1 = [cp.tile([P, E], F32, tag=f"w1_{q}", name=f"w1_{q}")
                  for q in range(3)]
            w0 = [cp.tile([P, E], F32, tag=f"w0_{q}", name=f"w0_{q}")
                  for q in range(3)]
            wh = [[cp.tile([P, E], F32, tag=f"wh{k}_{q}", name=f"wh{k}_{q}")
                   for q in range(3)] for k in range(3)]
            wd = [cp.tile([E, D], F32, tag=f"wd{k}", name=f"wdt{k}")
                  for k in range(3)]
            for q in range(3):
                nc.sync.dma_start(out=w1[q][:], in_=w1_d[q * P:(q + 1) * P, :])
                nc.sync.dma_start(out=w0[q][:], in_=w0_d[q * P:(q + 1) * P, :])
                for k in range(3):
                    nc.sync.dma_start(out=wh[k][q][:],
                                      in_=wh_d[k][q * P:(q + 1) * P, :])
            for k in range(3):
                nc.sync.dma_start(out=wd[k][:], in_=wd_d[k][:, :])

            x0 = ip.tile([BL * 2, 1], I32)
            nc.sync.dma_start(out=x0[:], in_=nodes2d)
            # base-node offsets padded to 128 partitions for indirect_copy
            x0p = ip.tile([P, 1], I32)
            nc.vector.memset(x0p[:], 0)
            nc.sync.dma_start(out=x0p[0:BL * 2, :], in_=nodes2d)

            # ---- hop-1 sample ids -----------------------------------------
            # r0[p, :] = nbrcat[nodes[p], :]
            r0 = ip.tile([P, 2 * MAX_DEGREE], I32)
            _gather_rows(nc, r0[:], nbrcat[:, :], x0p[:])
            # select the 50 sample columns per base node -> s1loc rows [0:64]
            s1loc = ip.tile([P, 50], I32)
            if USE_INDCOPY:
                sel1 = ip.tile([P, 4], U16)
                nc.sync.dma_start(out=sel1[:], in_=sel1_d[:, :])
                nc.gpsimd.indirect_copy(out=s1loc[:], data=r0[:],
                                        idxs=sel1[:],
                                        i_know_ap_gather_is_preferred=True)
            else:
                for s in range(2):
                    for j, col in enumerate(C1_BY_SIDE[s]):
                        nc.vector.tensor_copy(
                            out=s1loc[s * BL:(s + 1) * BL, j:j + 1],
                            in_=r0[s * BL:(s + 1) * BL, col:col + 1])
            # bounce through DRAM to repack [64, 50] -> [128, 25] node-major
            for s in range(2):
                nc.sync.dma_start(
                    out=s1_scr[s * NS1:(s + 1) * NS1].rearrange(
                        "(b j) -> b j", j=50),
                    in_=s1loc[s * BL:(s + 1) * BL, :])
            s1c = ip.tile([P, NT], I32)
            nc.sync.dma_start(out=s1c[:],
                              in_=s1_scr[:].rearrange("(t p) -> p t", p=P))

            # ---- hop-1 neighbor rows + hop-2 id selection -----------------
            rbuf = bp.tile([P, NT * 2 * MAX_DEGREE], I32, tag="rbuf")
            r3 = rbuf[:].rearrange("p (t c) -> p t c", c=2 * MAX_DEGREE)
            for t in range(NT):
                _gather_rows(nc, r3[:, t, :], nbrcat[:, :], s1c[:, t:t + 1])
            ids2 = ip.tile([P, NT * 20], I32)
            ids2v = ids2[:].rearrange("p (t j) -> p t j", j=20)
            if USE_INDCOPY:
                idsel = ip.tile([P, 32], U16)
                nc.sync.dma_start(out=idsel[:], in_=idsel_d[:, :])
                nc.gpsimd.indirect_copy(out=ids2[:], data=rbuf[:],
                                        idxs=idsel[:],
                                        i_know_ap_gather_is_preferred=True)
            else:
                for t in range(NT):
                    lo_side = 0 if (t * P) < NS1 else 1
                    hi_side = 0 if (t * P + P - 1) < NS1 else 1
                    if lo_side == hi_side:
                        for j, col in enumerate(C2_BY_SIDE[lo_side]):
                            nc.vector.tensor_copy(out=ids2v[:, t, j],
                                                  in_=r3[:, t, col])
                    else:
                        cut = NS1 - t * P
                        for j in range(20):
                            nc.vector.tensor_copy(
                                out=ids2v[0:cut, t, j],
                                in_=r3[0:cut, t, C2_BY_SIDE[0][j]])
                            nc.vector.tensor_copy(
                                out=ids2v[cut:P, t, j],
                                in_=r3[cut:P, t, C2_BY_SIDE[1][j]])

            # ---- per-tile pipeline ----------------------------------------
            fselfT = bp.tile([P, NT * P], F32, tag="fselfT")
            h1T = bp.tile([P, NT * P], F32, tag="h1T")

            for t in range(NT):
                # self features for this tile's 128 hop-1 nodes
                fs = gp.tile([P, F], F32, tag="fs")
                _gather_rows(nc, fs[:], feats[:, :], s1c[:, t:t + 1])
                # neighbor features, accumulated on DVE as they arrive
                acc_o = ap_.tile([P, F], F32, tag="acc_o")
                acc_i = ap_.tile([P, F], F32, tag="acc_i")
                for j in range(20):
                    g = gp.tile([P, F], F32, tag="g")
                    _gather_rows(nc, g[:], feats[:, :], ids2v[:, t, j:j + 1])
                    acc = acc_o if j < 10 else acc_i
                    if j % 10 == 0:
                        nc.vector.tensor_copy(out=acc[:], in_=g[:])
                    else:
                        nc.vector.tensor_add(out=acc[:], in0=acc[:], in1=g[:])

                # transpose self + neighbor sums to feature-major via PE
                ps_s = pa.tile([P, P], F32, tag="ps_s", space="PSUM")
                ps_o = pa.tile([P, P], F32, tag="ps_o", space="PSUM")
                ps_i = pa.tile([P, P], F32, tag="ps_i", space="PSUM")
                nc.tensor.matmul(out=ps_s[:], lhsT=fs[:], rhs=ident[:],
                                 start=True, stop=True, is_transpose=True)
                nc.tensor.matmul(out=ps_o[:], lhsT=acc_o[:], rhs=ident[:],
                                 start=True, stop=True, is_transpose=True)
                nc.tensor.matmul(out=ps_i[:], lhsT=acc_i[:], rhs=ident[:],
                                 start=True, stop=True, is_transpose=True)
                so = fp.tile([P, P], F32, tag="so")
                si = fp.tile([P, P], F32, tag="si")
                nc.vector.tensor_copy(out=so[:], in_=ps_o[:])
                nc.vector.tensor_copy(out=si[:], in_=ps_i[:])
                nc.vector.tensor_copy(out=fselfT[:, t * P:(t + 1) * P],
                                      in_=ps_s[:])

                ph = pm.tile([P, P], F32, tag="ph", space="PSUM")
                nc.tensor.matmul(out=ph[:], lhsT=w1[0][:],
                                 rhs=fselfT[:, t * P:(t + 1) * P],
                                 start=True, stop=False)
                nc.tensor.matmul(out=ph[:], lhsT=w1[1][:], rhs=so[:],
                                 start=False, stop=False)
                nc.tensor.matmul(out=ph[:], lhsT=w1[2][:], rhs=si[:],
                                 start=False, stop=True)
                nc.scalar.activation(out=h1T[:, t * P:(t + 1) * P], in_=ph[:],
                                     func=SIG)

            # ---- hop-0 GC --------------------------------------------------
            fb = sp_.tile([BL * 2, F], F32, tag="fb")
            _gather_rows(nc, fb[:], feats[:, :], x0[:])
            ps_fbT = pa.tile([P, BL * 2], F32, tag="ps_s", space="PSUM")
            nc.tensor.matmul(out=ps_fbT[:], lhsT=fb[:],
                             rhs=ident[:BL * 2, :BL * 2], start=True,
                             stop=True, is_transpose=True)
            fbT = sp_.tile([P, BL * 2], F32, tag="fbT")
            nc.vector.tensor_copy(out=fbT[:], in_=ps_fbT[:])

            h0T = []
            for s in range(2):
                m0 = fp.tile([P, BL * 2], F32, tag="m0")
                m0v = m0[:].rearrange("p (b h) -> p b h", h=2)
                view = fselfT[:, NS1 * s:NS1 * (s + 1)].rearrange(
                    "p (b h j) -> p b h j", h=2, j=25)
                nc.vector.tensor_reduce(out=m0v, in_=view,
                                        axis=mybir.AxisListType.X,
                                        op=mybir.AluOpType.add)
                ph0 = pm.tile([P, BL], F32, tag="ph", space="PSUM")
                nc.tensor.matmul(out=ph0[:], lhsT=w0[0][:],
                                 rhs=fbT[:, s * BL:(s + 1) * BL],
                                 start=True, stop=False)
                nc.tensor.matmul(out=ph0[:], lhsT=w0[1][:], rhs=m0v[:, :, 0],
                                 start=False, stop=False)
                nc.tensor.matmul(out=ph0[:], lhsT=w0[2][:], rhs=m0v[:, :, 1],
                                 start=False, stop=True)
                h0 = sp_.tile([P, BL], F32, tag=f"h0_{s}", name=f"h0_{s}")
                nc.scalar.activation(out=h0[:], in_=ph0[:], func=SIG)
                h0T.append(h0)

            # ---- layer-1 heads + final projection -------------------------
            for s in range(2):
                mh = fp.tile([P, BL * 2], F32, tag="mh")
                mhv = mh[:].rearrange("p (b h) -> p b h", h=2)
                view = h1T[:, NS1 * s:NS1 * (s + 1)].rearrange(
                    "p (b h j) -> p b h j", h=2, j=25)
                nc.vector.tensor_reduce(out=mhv, in_=view,
                                        axis=mybir.AxisListType.X,
                                        op=mybir.AluOpType.add)
                for k in range(3):
                    pz = pm.tile([P, BL], F32, tag="ph", space="PSUM")
                    nc.tensor.matmul(out=pz[:], lhsT=wh[k][0][:],
                                     rhs=h0T[s][:], start=True, stop=False)
                    nc.tensor.matmul(out=pz[:], lhsT=wh[k][1][:],
                                     rhs=mhv[:, :, 0], start=False, stop=False)
                    nc.tensor.matmul(out=pz[:], lhsT=wh[k][2][:],
                                     rhs=mhv[:, :, 1], start=False, stop=True)
                    zt = fp.tile([P, BL], F32, tag="zt")
                    nc.scalar.activation(out=zt[:], in_=pz[:], func=SIG)
                    po = pm.tile([D, BL], F32, tag="po", space="PSUM")
                    nc.tensor.matmul(out=po[:], lhsT=wd[k][:], rhs=zt[:],
                                     start=True, stop=True)
                    ot = fp.tile([D, BL], F32, tag="ot")
                    nc.vector.tensor_copy(out=ot[:], in_=po[:])
                    nc.sync.dma_start(out=out_d[s * 3 + k, :, :], in_=ot[:])

    nc.compile()
    return nc


_NC_CACHE = None


def _get_nc():
    global _NC_CACHE
    if _NC_CACHE is None:
        _NC_CACHE = build_program()
    return _NC_CACHE


def host_prep(nodes1, nodes2, neighbors_out, neighbors_in, features,
              W_in, W_mean, W_std, W_pi, Wd_mean, Wd_std, Wd_pi):
    nodes1 = np.asarray(nodes1, dtype=np.int32)
    nodes2 = np.asarray(nodes2, dtype=np.int32)
    nbrcat = np.ascontiguousarray(np.concatenate(
        [np.asarray(neighbors_out, dtype=np.int32),
         np.asarray(neighbors_in, dtype=np.int32)], axis=1))
    features = np.ascontiguousarray(np.asarray(features, dtype=np.float32))

    def scale(w, f):
        w = np.array(w, dtype=np.float32, copy=True)
        w[F:] *= np.float32(1.0 / f)
        return w

    w1 = scale(W_in, 10.0)
    w0 = scale(W_in, 25.0)
    whs = [scale(W_mean, 25.0), scale(W_std, 25.0), scale(W_pi, 25.0)]
    wds = [np.ascontiguousarray(np.asarray(w, dtype=np.float32))
           for w in (Wd_mean, Wd_std, Wd_pi)]
    sel1, idsel = _host_consts()

    in_maps = []
    for c in range(NCORES):
        nloc = np.ascontiguousarray(np.concatenate(
            [nodes1[c * BL:(c + 1) * BL], nodes2[c * BL:(c + 1) * BL]]))
        m = {"nodes": nloc, "nbrcat": nbrcat, "feats": features,
             "sel1": sel1, "idsel": idsel, "w1": w1, "w0": w0}
        for k in range(3):
            m[f"wh{k}"] = whs[k]
            m[f"wd{k}"] = wds[k]
        in_maps.append(m)
    return in_maps


def kernel(nodes1, nodes2, neighbors_out, neighbors_in, features,
           W_in, W_mean, W_std, W_pi, W_ag, W_ad, Wd_mean, Wd_std, Wd_pi,
           _trace=False):
    in_maps = host_prep(nodes1, nodes2, neighbors_out, neighbors_in, features,
                        W_in, W_mean, W_std, W_pi, Wd_mean, Wd_std, Wd_pi)
    nc = _get_nc()
    from concourse.bass_utils import run_bass_kernel_spmd
    res = run_bass_kernel_spmd(nc, in_maps, list(range(NCORES)),
                               trace=_trace)
    if _trace:
        kernel.last_results = res

    out = np.zeros((6, B, D), dtype=np.float32)
    for c in range(NCORES):
        o = res.results[c]["out"]  # [6, D, BL]
        for i in range(6):
            out[i, c * BL:(c + 1) * BL, :] = o[i].T
    return out



# revision 7
# speedup vs baseline: 1.2445x; 1.0752x over previous
"""Trainium2 Bass kernel for the DLSM GNN message-passing model.

Data-parallel over the batch: each of the 8 NeuronCores handles 32 nodes of
nodes1 + 32 nodes of nodes2; feature table and weights are replicated per
core.

Hardware contract: indirect DMA gathers one arbitrary row per partition per
instruction ([128,1] offsets). The kernel is organized as a stream of
[128, row] gathers on the GPSIMD/SWDGE engine with all other work (DVE
accumulation of neighbor sums, PE transposes + GC projections, ACT sigmoids,
DVE strided reduces for the hop-0/layer-1 means) overlapped underneath it.

The hop-1/hop-2 sample ids are a deterministic function of the adjacency
tables and the reference's fixed jax.random.key(42) sampling columns, so
they are computed host-side (verified exactly equal to the reference's
_sample output) and shipped as two small int32 index tables. This removes
the on-device id-derivation chain (base-row gather, indirect_copy column
selects, DRAM repack bounce, 25 neighbor-table row gathers) that serialized
the head of the pipeline; every feature gather's offsets are resident in
SBUF after two input DMAs, so the 525-instruction gather stream issues
dependency-free. GC mean factors are folded into host-prescaled weights.
"""
import os
import sys
import numpy as np

sys.path.insert(0, '/opt/trn_rl_repo')

import concourse.bass as bass  # noqa: E402
import concourse.tile as tile  # noqa: E402
from concourse import bacc, mybir  # noqa: E402
from concourse.masks import make_identity  # noqa: E402

# ---- problem constants -----------------------------------------------------
N = 200000
F = 128
B = 256
E = 128
D = 64
MAX_DEGREE = 64
NCORES = 8
BL = B // NCORES          # base nodes per core per side (32)
NB = 2 * BL               # base nodes per core (64)
NS1 = BL * 50             # hop-1 samples per side (1600)
NS = 2 * NS1              # hop-1 samples per core (3200)
NT = NS // 128            # 25 tiles of 128 hop-1 nodes (both sides)
P = 128

SINGLE_PACKET = os.environ.get('K_SP', '0') == '1'

# Sampling columns fixed by jax.random.key(42) inside the reference.
S1_C1_OUT = [10, 56, 8, 17, 28, 26, 9, 20, 22, 35, 15, 4, 14, 21, 6, 53, 27,
             47, 49, 46, 41, 13, 63, 38, 54]
S1_C1_IN = [19, 59, 37, 12, 34, 31, 29, 1, 3, 0, 24, 40, 26, 11, 25, 23, 13,
            27, 43, 6, 57, 35, 58, 51, 9]
S1_C2_OUT = [57, 36, 9, 2, 34, 3, 6, 11, 0, 21]
S1_C2_IN = [33, 13, 21, 0, 54, 16, 46, 24, 30, 43]
S2_C1_OUT = [9, 7, 34, 52, 15, 35, 54, 30, 10, 16, 42, 56, 51, 28, 12, 19,
             24, 49, 2, 38, 43, 32, 48, 1, 39]
S2_C1_IN = [53, 47, 39, 57, 37, 27, 4, 20, 36, 31, 60, 38, 12, 43, 3, 21, 25,
            58, 48, 52, 23, 35, 15, 28, 7]
S2_C2_OUT = [41, 25, 9, 57, 45, 62, 42, 37, 31, 63]
S2_C2_IN = [40, 34, 60, 56, 2, 14, 6, 32, 50, 25]

F32 = mybir.dt.float32
I32 = mybir.dt.int32
SIG = mybir.ActivationFunctionType.Sigmoid


def _gather_rows(nc, out_ap, table_ap, off_ap):
    """One [128,1]-offset indirect row gather (the HW-verified contract)."""
    inst = nc.gpsimd.indirect_dma_start(
        out=out_ap, out_offset=None, in_=table_ap,
        in_offset=bass.IndirectOffsetOnAxis(ap=off_ap, axis=0))
    if SINGLE_PACKET:
        inst.ins.single_packet = True
    return inst


def build_program():
    nc = bacc.Bacc("TRN2", target_bir_lowering=False, debug=False)

    nodes = nc.dram_tensor("nodes", [NB], I32, kind="ExternalInput")
    feats = nc.dram_tensor("feats", [N, F], F32, kind="ExternalInput")
    s1c_d = nc.dram_tensor("s1c", [P, NT], I32, kind="ExternalInput")
    ids2_d = nc.dram_tensor("ids2", [P, NT * 20], I32, kind="ExternalInput")
    w1_d = nc.dram_tensor("w1", [3 * F, E], F32, kind="ExternalInput")
    w0_d = nc.dram_tensor("w0", [3 * F, E], F32, kind="ExternalInput")
    wh_d = [nc.dram_tensor(f"wh{k}", [3 * E, E], F32, kind="ExternalInput")
            for k in range(3)]
    wd_d = [nc.dram_tensor(f"wd{k}", [E, D], F32, kind="ExternalInput")
            for k in range(3)]
    out_d = nc.dram_tensor("out", [6, D, BL], F32, kind="ExternalOutput")

    nodes2d = nodes[:].rearrange("(n o) -> n o", o=1)

    with tile.TileContext(nc) as tc:
        with (
            tc.tile_pool(name="const", bufs=1) as cp,
            tc.tile_pool(name="ids", bufs=1) as ip,
            tc.tile_pool(name="big", bufs=1) as bp,
            tc.tile_pool(name="g", bufs=8) as gp,
            tc.tile_pool(name="acc", bufs=3) as ap_,
            tc.tile_pool(name="fmaj", bufs=4) as fp,
            tc.tile_pool(name="small", bufs=2) as sp_,
            tc.tile_pool(name="ps_acc", bufs=2, space="PSUM") as pa,
            tc.tile_pool(name="ps_mm", bufs=1, space="PSUM") as pm,
        ):
            # ---- constants + index tables ---------------------------------
            ident = cp.tile([P, P], F32)
            make_identity(nc, ident[:])

            w1 = [cp.tile([P, E], F32, tag=f"w1_{q}", name=f"w1_{q}")
                  for q in range(3)]
            w0 = [cp.tile([P, E], F32, tag=f"w0_{q}", name=f"w0_{q}")
                  for q in range(3)]
            wh = [[cp.tile([P, E], F32, tag=f"wh{k}_{q}", name=f"wh{k}_{q}")
                   for q in range(3)] for k in range(3)]
            wd = [cp.tile([E, D], F32, tag=f"wd{k}", name=f"wdt{k}")
                  for k in range(3)]
            for q in range(3):
                nc.sync.dma_start(out=w1[q][:], in_=w1_d[q * P:(q + 1) * P, :])
                nc.sync.dma_start(out=w0[q][:], in_=w0_d[q * P:(q + 1) * P, :])
                for k in range(3):
                    nc.sync.dma_start(out=wh[k][q][:],
                                      in_=wh_d[k][q * P:(q + 1) * P, :])
            for k in range(3):
                nc.sync.dma_start(out=wd[k][:], in_=wd_d[k][:, :])

            x0 = ip.tile([NB, 1], I32)
            nc.sync.dma_start(out=x0[:], in_=nodes2d)
            s1c = ip.tile([P, NT], I32)
            nc.sync.dma_start(out=s1c[:], in_=s1c_d[:, :])
            ids2 = ip.tile([P, NT * 20], I32)
            nc.sync.dma_start(out=ids2[:], in_=ids2_d[:, :])
            ids2v = ids2[:].rearrange("p (t j) -> p t j", j=20)

            # ---- per-tile pipeline ----------------------------------------
            fselfT = bp.tile([P, NT * P], F32, tag="fselfT")
            h1T = bp.tile([P, NT * P], F32, tag="h1T")

            for t in range(NT):
                # self features for this tile's 128 hop-1 nodes
                fs = gp.tile([P, F], F32, tag="fs")
                _gather_rows(nc, fs[:], feats[:, :], s1c[:, t:t + 1])
                # neighbor features, accumulated on DVE as they arrive
                acc_o = ap_.tile([P, F], F32, tag="acc_o")
                acc_i = ap_.tile([P, F], F32, tag="acc_i")
                for j in range(20):
                    g = gp.tile([P, F], F32, tag="g")
                    _gather_rows(nc, g[:], feats[:, :], ids2v[:, t, j:j + 1])
                    acc = acc_o if j < 10 else acc_i
                    if j % 10 == 0:
                        nc.vector.tensor_copy(out=acc[:], in_=g[:])
                    else:
                        nc.vector.tensor_add(out=acc[:], in0=acc[:], in1=g[:])

                # transpose self + neighbor sums to feature-major via PE
                ps_s = pa.tile([P, P], F32, tag="ps_s", space="PSUM")
                ps_o = pa.tile([P, P], F32, tag="ps_o", space="PSUM")
                ps_i = pa.tile([P, P], F32, tag="ps_i", space="PSUM")
                nc.tensor.matmul(out=ps_s[:], lhsT=fs[:], rhs=ident[:],
                                 start=True, stop=True, is_transpose=True)
                nc.tensor.matmul(out=ps_o[:], lhsT=acc_o[:], rhs=ident[:],
                                 start=True, stop=True, is_transpose=True)
                nc.tensor.matmul(out=ps_i[:], lhsT=acc_i[:], rhs=ident[:],
                                 start=True, stop=True, is_transpose=True)
                so = fp.tile([P, P], F32, tag="so")
                si = fp.tile([P, P], F32, tag="si")
                nc.vector.tensor_copy(out=so[:], in_=ps_o[:])
                nc.vector.tensor_copy(out=si[:], in_=ps_i[:])
                nc.vector.tensor_copy(out=fselfT[:, t * P:(t + 1) * P],
                                      in_=ps_s[:])

                ph = pm.tile([P, P], F32, tag="ph", space="PSUM")
                nc.tensor.matmul(out=ph[:], lhsT=w1[0][:],
                                 rhs=fselfT[:, t * P:(t + 1) * P],
                                 start=True, stop=False)
                nc.tensor.matmul(out=ph[:], lhsT=w1[1][:], rhs=so[:],
                                 start=False, stop=False)
                nc.tensor.matmul(out=ph[:], lhsT=w1[2][:], rhs=si[:],
                                 start=False, stop=True)
                nc.scalar.activation(out=h1T[:, t * P:(t + 1) * P], in_=ph[:],
                                     func=SIG)

            # ---- hop-0 GC --------------------------------------------------
            fb = sp_.tile([NB, F], F32, tag="fb")
            _gather_rows(nc, fb[:], feats[:, :], x0[:])
            ps_fbT = pa.tile([P, NB], F32, tag="ps_s", space="PSUM")
            nc.tensor.matmul(out=ps_fbT[:], lhsT=fb[:],
                             rhs=ident[:NB, :NB], start=True,
                             stop=True, is_transpose=True)
            fbT = sp_.tile([P, NB], F32, tag="fbT")
            nc.vector.tensor_copy(out=fbT[:], in_=ps_fbT[:])

            h0T = []
            for s in range(2):
                m0 = fp.tile([P, NB], F32, tag="m0")
                m0v = m0[:].rearrange("p (b h) -> p b h", h=2)
                view = fselfT[:, NS1 * s:NS1 * (s + 1)].rearrange(
                    "p (b h j) -> p b h j", h=2, j=25)
                nc.vector.tensor_reduce(out=m0v, in_=view,
                                        axis=mybir.AxisListType.X,
                                        op=mybir.AluOpType.add)
                ph0 = pm.tile([P, BL], F32, tag="ph", space="PSUM")
                nc.tensor.matmul(out=ph0[:], lhsT=w0[0][:],
                                 rhs=fbT[:, s * BL:(s + 1) * BL],
                                 start=True, stop=False)
                nc.tensor.matmul(out=ph0[:], lhsT=w0[1][:], rhs=m0v[:, :, 0],
                                 start=False, stop=False)
                nc.tensor.matmul(out=ph0[:], lhsT=w0[2][:], rhs=m0v[:, :, 1],
                                 start=False, stop=True)
                h0 = sp_.tile([P, BL], F32, tag=f"h0_{s}", name=f"h0_{s}")
                nc.scalar.activation(out=h0[:], in_=ph0[:], func=SIG)
                h0T.append(h0)

            # ---- layer-1 heads + final projection -------------------------
            for s in range(2):
                mh = fp.tile([P, NB], F32, tag="mh")
                mhv = mh[:].rearrange("p (b h) -> p b h", h=2)
                view = h1T[:, NS1 * s:NS1 * (s + 1)].rearrange(
                    "p (b h j) -> p b h j", h=2, j=25)
                nc.vector.tensor_reduce(out=mhv, in_=view,
                                        axis=mybir.AxisListType.X,
                                        op=mybir.AluOpType.add)
                for k in range(3):
                    pz = pm.tile([P, BL], F32, tag="ph", space="PSUM")
                    nc.tensor.matmul(out=pz[:], lhsT=wh[k][0][:],
                                     rhs=h0T[s][:], start=True, stop=False)
                    nc.tensor.matmul(out=pz[:], lhsT=wh[k][1][:],
                                     rhs=mhv[:, :, 0], start=False, stop=False)
                    nc.tensor.matmul(out=pz[:], lhsT=wh[k][2][:],
                                     rhs=mhv[:, :, 1], start=False, stop=True)
                    zt = fp.tile([P, BL], F32, tag="zt")
                    nc.scalar.activation(out=zt[:], in_=pz[:], func=SIG)
                    po = pm.tile([D, BL], F32, tag="po", space="PSUM")
                    nc.tensor.matmul(out=po[:], lhsT=wd[k][:], rhs=zt[:],
                                     start=True, stop=True)
                    ot = fp.tile([D, BL], F32, tag="ot")
                    nc.vector.tensor_copy(out=ot[:], in_=po[:])
                    nc.sync.dma_start(out=out_d[s * 3 + k, :, :], in_=ot[:])

    nc.compile()
    return nc


_NC_CACHE = None


def _get_nc():
    global _NC_CACHE
    if _NC_CACHE is None:
        _NC_CACHE = build_program()
    return _NC_CACHE


def _core_indices(nodes1c, nodes2c, nb_out, nb_in):
    """Host-side replica of the reference sampling for one core.
    Returns (nloc [64] i32, s1c [128, 25] i32, ids2 [128, 500] i32)."""
    nloc = np.concatenate([nodes1c, nodes2c]).astype(np.int64)  # [64]
    s1 = np.empty((NB, 50), dtype=np.int64)
    s1[:BL] = np.concatenate([nb_out[nloc[:BL]][:, S1_C1_OUT],
                              nb_in[nloc[:BL]][:, S1_C1_IN]], axis=1)
    s1[BL:] = np.concatenate([nb_out[nloc[BL:]][:, S2_C1_OUT],
                              nb_in[nloc[BL:]][:, S2_C1_IN]], axis=1)
    s1f = s1.reshape(-1)  # sample i = base*50 + j; tile slot: i = t*128 + p
    ids2 = np.empty((NS, 20), dtype=np.int64)
    ids2[:NS1] = np.concatenate([nb_out[s1f[:NS1]][:, S1_C2_OUT],
                                 nb_in[s1f[:NS1]][:, S1_C2_IN]], axis=1)
    ids2[NS1:] = np.concatenate([nb_out[s1f[NS1:]][:, S2_C2_OUT],
                                 nb_in[s1f[NS1:]][:, S2_C2_IN]], axis=1)
    s1c = np.ascontiguousarray(
        s1f.reshape(NT, P).T.astype(np.int32))              # [128, 25]
    ids2h = np.ascontiguousarray(
        ids2.reshape(NT, P, 20).transpose(1, 0, 2)
        .reshape(P, NT * 20).astype(np.int32))              # [128, 500]
    return nloc.astype(np.int32), s1c, ids2h


def host_prep(nodes1, nodes2, neighbors_out, neighbors_in, features,
              W_in, W_mean, W_std, W_pi, Wd_mean, Wd_std, Wd_pi):
    nodes1 = np.asarray(nodes1, dtype=np.int64)
    nodes2 = np.asarray(nodes2, dtype=np.int64)
    nb_out = np.asarray(neighbors_out, dtype=np.int64)
    nb_in = np.asarray(neighbors_in, dtype=np.int64)
    features = np.ascontiguousarray(np.asarray(features, dtype=np.float32))

    def scale(w, f):
        w = np.array(w, dtype=np.float32, copy=True)
        w[F:] *= np.float32(1.0 / f)
        return w

    w1 = scale(W_in, 10.0)
    w0 = scale(W_in, 25.0)
    whs = [scale(W_mean, 25.0), scale(W_std, 25.0), scale(W_pi, 25.0)]
    wds = [np.ascontiguousarray(np.asarray(w, dtype=np.float32))
           for w in (Wd_mean, Wd_std, Wd_pi)]

    in_maps = []
    for c in range(NCORES):
        nloc, s1c, ids2h = _core_indices(nodes1[c * BL:(c + 1) * BL],
                                         nodes2[c * BL:(c + 1) * BL],
                                         nb_out, nb_in)
        m = {"nodes": nloc, "feats": features, "s1c": s1c, "ids2": ids2h,
             "w1": w1, "w0": w0}
        for k in range(3):
            m[f"wh{k}"] = whs[k]
            m[f"wd{k}"] = wds[k]
        in_maps.append(m)
    return in_maps


def kernel(nodes1, nodes2, neighbors_out, neighbors_in, features,
           W_in, W_mean, W_std, W_pi, W_ag, W_ad, Wd_mean, Wd_std, Wd_pi,
           _trace=False):
    in_maps = host_prep(nodes1, nodes2, neighbors_out, neighbors_in, features,
                        W_in, W_mean, W_std, W_pi, Wd_mean, Wd_std, Wd_pi)
    nc = _get_nc()
    from concourse.bass_utils import run_bass_kernel_spmd
    res = run_bass_kernel_spmd(nc, in_maps, list(range(NCORES)),
                               trace=_trace)
    if _trace:
        kernel.last_results = res

    out = np.zeros((6, B, D), dtype=np.float32)
    for c in range(NCORES):
        o = res.results[c]["out"]  # [6, D, BL]
        for i in range(6):
            out[i, c * BL:(c + 1) * BL, :] = o[i].T
    return out


# revision 8
# speedup vs baseline: 1.2598x; 1.0123x over previous
"""Trainium2 Bass kernel for the DLSM GNN message-passing model.

Data-parallel over the batch: each of the 8 NeuronCores handles 32 nodes of
nodes1 + 32 nodes of nodes2; feature table and weights are replicated per
core.

Hardware contract: indirect DMA gathers one arbitrary row per partition per
instruction ([128,1] offsets). The kernel is organized as a stream of
[128, row] gathers on the GPSIMD/SWDGE engine with all other work (DVE
accumulation of neighbor sums, PE transposes + GC projections, ACT sigmoids,
DVE strided reduces for the hop-0/layer-1 means) overlapped underneath it.

The hop-1/hop-2 sample ids are a deterministic function of the adjacency
tables and the reference's fixed jax.random.key(42) sampling columns, so
they are computed host-side (verified exactly equal to the reference's
_sample output) and shipped as two small int32 index tables. This removes
the on-device id-derivation chain (base-row gather, indirect_copy column
selects, DRAM repack bounce, 25 neighbor-table row gathers) that serialized
the head of the pipeline; every feature gather's offsets are resident in
SBUF after two input DMAs, so the 525-instruction gather stream issues
dependency-free. GC mean factors are folded into host-prescaled weights.
"""
import os
import sys
import numpy as np

sys.path.insert(0, '/opt/trn_rl_repo')

import concourse.bass as bass  # noqa: E402
import concourse.tile as tile  # noqa: E402
from concourse import bacc, mybir  # noqa: E402
from concourse.masks import make_identity  # noqa: E402

# ---- problem constants -----------------------------------------------------
N = 200000
F = 128
B = 256
E = 128
D = 64
MAX_DEGREE = 64
NCORES = 8
BL = B // NCORES          # base nodes per core per side (32)
NB = 2 * BL               # base nodes per core (64)
NS1 = BL * 50             # hop-1 samples per side (1600)
NS = 2 * NS1              # hop-1 samples per core (3200)
NT = NS // 128            # 25 tiles of 128 hop-1 nodes (both sides)
P = 128

SINGLE_PACKET = os.environ.get('K_SP', '0') == '1'

# Sampling columns fixed by jax.random.key(42) inside the reference.
S1_C1_OUT = [10, 56, 8, 17, 28, 26, 9, 20, 22, 35, 15, 4, 14, 21, 6, 53, 27,
             47, 49, 46, 41, 13, 63, 38, 54]
S1_C1_IN = [19, 59, 37, 12, 34, 31, 29, 1, 3, 0, 24, 40, 26, 11, 25, 23, 13,
            27, 43, 6, 57, 35, 58, 51, 9]
S1_C2_OUT = [57, 36, 9, 2, 34, 3, 6, 11, 0, 21]
S1_C2_IN = [33, 13, 21, 0, 54, 16, 46, 24, 30, 43]
S2_C1_OUT = [9, 7, 34, 52, 15, 35, 54, 30, 10, 16, 42, 56, 51, 28, 12, 19,
             24, 49, 2, 38, 43, 32, 48, 1, 39]
S2_C1_IN = [53, 47, 39, 57, 37, 27, 4, 20, 36, 31, 60, 38, 12, 43, 3, 21, 25,
            58, 48, 52, 23, 35, 15, 28, 7]
S2_C2_OUT = [41, 25, 9, 57, 45, 62, 42, 37, 31, 63]
S2_C2_IN = [40, 34, 60, 56, 2, 14, 6, 32, 50, 25]

F32 = mybir.dt.float32
I32 = mybir.dt.int32
SIG = mybir.ActivationFunctionType.Sigmoid


def _gather_rows(nc, out_ap, table_ap, off_ap):
    """One [128,1]-offset indirect row gather (the HW-verified contract)."""
    inst = nc.gpsimd.indirect_dma_start(
        out=out_ap, out_offset=None, in_=table_ap,
        in_offset=bass.IndirectOffsetOnAxis(ap=off_ap, axis=0))
    if SINGLE_PACKET:
        inst.ins.single_packet = True
    return inst


def build_program():
    nc = bacc.Bacc("TRN2", target_bir_lowering=False, debug=False)

    nodes = nc.dram_tensor("nodes", [NB], I32, kind="ExternalInput")
    feats = nc.dram_tensor("feats", [N, F], F32, kind="ExternalInput")
    s1c_d = nc.dram_tensor("s1c", [P, NT], I32, kind="ExternalInput")
    ids2_d = nc.dram_tensor("ids2", [P, NT * 20], I32, kind="ExternalInput")
    w1_d = nc.dram_tensor("w1", [3 * F, E], F32, kind="ExternalInput")
    w0_d = nc.dram_tensor("w0", [3 * F, E], F32, kind="ExternalInput")
    wh_d = [nc.dram_tensor(f"wh{k}", [3 * E, E], F32, kind="ExternalInput")
            for k in range(3)]
    wd_d = [nc.dram_tensor(f"wd{k}", [E, D], F32, kind="ExternalInput")
            for k in range(3)]
    out_d = nc.dram_tensor("out", [6, D, BL], F32, kind="ExternalOutput")

    nodes2d = nodes[:].rearrange("(n o) -> n o", o=1)

    with tile.TileContext(nc) as tc:
        with (
            tc.tile_pool(name="const", bufs=1) as cp,
            tc.tile_pool(name="ids", bufs=1) as ip,
            tc.tile_pool(name="big", bufs=1) as bp,
            tc.tile_pool(name="g", bufs=8) as gp,
            tc.tile_pool(name="acc", bufs=3) as ap_,
            tc.tile_pool(name="fmaj", bufs=4) as fp,
            tc.tile_pool(name="small", bufs=2) as sp_,
            tc.tile_pool(name="ps_acc", bufs=2, space="PSUM") as pa,
            tc.tile_pool(name="ps_mm", bufs=1, space="PSUM") as pm,
        ):
            # ---- constants + index tables ---------------------------------
            ident = cp.tile([P, P], F32)
            make_identity(nc, ident[:])

            x0 = ip.tile([NB, 1], I32)
            nc.sync.dma_start(out=x0[:], in_=nodes2d)
            s1c = ip.tile([P, NT], I32)
            nc.sync.dma_start(out=s1c[:], in_=s1c_d[:, :])
            ids2 = ip.tile([P, NT * 20], I32)
            nc.sync.dma_start(out=ids2[:], in_=ids2_d[:, :])
            ids2v = ids2[:].rearrange("p (t j) -> p t j", j=20)

            w1 = [cp.tile([P, E], F32, tag=f"w1_{q}", name=f"w1_{q}")
                  for q in range(3)]
            w0 = [cp.tile([P, E], F32, tag=f"w0_{q}", name=f"w0_{q}")
                  for q in range(3)]
            wh = [[cp.tile([P, E], F32, tag=f"wh{k}_{q}", name=f"wh{k}_{q}")
                   for q in range(3)] for k in range(3)]
            wd = [cp.tile([E, D], F32, tag=f"wd{k}", name=f"wdt{k}")
                  for k in range(3)]
            for q in range(3):
                nc.sync.dma_start(out=w1[q][:], in_=w1_d[q * P:(q + 1) * P, :])
                nc.sync.dma_start(out=w0[q][:], in_=w0_d[q * P:(q + 1) * P, :])
                for k in range(3):
                    nc.sync.dma_start(out=wh[k][q][:],
                                      in_=wh_d[k][q * P:(q + 1) * P, :])
            for k in range(3):
                nc.sync.dma_start(out=wd[k][:], in_=wd_d[k][:, :])

            # hop-0 base features: gather + transpose up front
            fb = sp_.tile([NB, F], F32, tag="fb")
            _gather_rows(nc, fb[:], feats[:, :], x0[:])
            ps_fbT = pa.tile([P, NB], F32, tag="ps_s", space="PSUM")
            nc.tensor.matmul(out=ps_fbT[:], lhsT=fb[:],
                             rhs=ident[:NB, :NB], start=True,
                             stop=True, is_transpose=True)
            fbT = sp_.tile([P, NB], F32, tag="fbT")
            nc.vector.tensor_copy(out=fbT[:], in_=ps_fbT[:])

            # ---- per-tile pipeline ----------------------------------------
            fselfT = bp.tile([P, NT * P], F32, tag="fselfT")
            h1T = bp.tile([P, NT * P], F32, tag="h1T")

            for t in range(NT):
                # self features for this tile's 128 hop-1 nodes
                fs = gp.tile([P, F], F32, tag="fs")
                _gather_rows(nc, fs[:], feats[:, :], s1c[:, t:t + 1])
                # neighbor features, accumulated on DVE as they arrive
                acc_o = ap_.tile([P, F], F32, tag="acc_o")
                acc_i = ap_.tile([P, F], F32, tag="acc_i")
                for j in range(20):
                    g = gp.tile([P, F], F32, tag="g")
                    _gather_rows(nc, g[:], feats[:, :], ids2v[:, t, j:j + 1])
                    acc = acc_o if j < 10 else acc_i
                    if j % 10 == 0:
                        nc.vector.tensor_copy(out=acc[:], in_=g[:])
                    else:
                        nc.vector.tensor_add(out=acc[:], in0=acc[:], in1=g[:])

                # transpose self + neighbor sums to feature-major via PE
                ps_s = pa.tile([P, P], F32, tag="ps_s", space="PSUM")
                ps_o = pa.tile([P, P], F32, tag="ps_o", space="PSUM")
                ps_i = pa.tile([P, P], F32, tag="ps_i", space="PSUM")
                nc.tensor.matmul(out=ps_s[:], lhsT=fs[:], rhs=ident[:],
                                 start=True, stop=True, is_transpose=True)
                nc.tensor.matmul(out=ps_o[:], lhsT=acc_o[:], rhs=ident[:],
                                 start=True, stop=True, is_transpose=True)
                nc.tensor.matmul(out=ps_i[:], lhsT=acc_i[:], rhs=ident[:],
                                 start=True, stop=True, is_transpose=True)
                so = fp.tile([P, P], F32, tag="so")
                si = fp.tile([P, P], F32, tag="si")
                nc.vector.tensor_copy(out=so[:], in_=ps_o[:])
                nc.vector.tensor_copy(out=si[:], in_=ps_i[:])
                nc.vector.tensor_copy(out=fselfT[:, t * P:(t + 1) * P],
                                      in_=ps_s[:])

                ph = pm.tile([P, P], F32, tag="ph", space="PSUM")
                nc.tensor.matmul(out=ph[:], lhsT=w1[0][:],
                                 rhs=fselfT[:, t * P:(t + 1) * P],
                                 start=True, stop=False)
                nc.tensor.matmul(out=ph[:], lhsT=w1[1][:], rhs=so[:],
                                 start=False, stop=False)
                nc.tensor.matmul(out=ph[:], lhsT=w1[2][:], rhs=si[:],
                                 start=False, stop=True)
                nc.scalar.activation(out=h1T[:, t * P:(t + 1) * P], in_=ph[:],
                                     func=SIG)

            # ---- hop-0 GC --------------------------------------------------
            h0T = []
            for s in range(2):
                m0 = fp.tile([P, NB], F32, tag="m0")
                m0v = m0[:].rearrange("p (b h) -> p b h", h=2)
                view = fselfT[:, NS1 * s:NS1 * (s + 1)].rearrange(
                    "p (b h j) -> p b h j", h=2, j=25)
                nc.vector.tensor_reduce(out=m0v, in_=view,
                                        axis=mybir.AxisListType.X,
                                        op=mybir.AluOpType.add)
                ph0 = pm.tile([P, BL], F32, tag="ph", space="PSUM")
                nc.tensor.matmul(out=ph0[:], lhsT=w0[0][:],
                                 rhs=fbT[:, s * BL:(s + 1) * BL],
                                 start=True, stop=False)
                nc.tensor.matmul(out=ph0[:], lhsT=w0[1][:], rhs=m0v[:, :, 0],
                                 start=False, stop=False)
                nc.tensor.matmul(out=ph0[:], lhsT=w0[2][:], rhs=m0v[:, :, 1],
                                 start=False, stop=True)
                h0 = sp_.tile([P, BL], F32, tag=f"h0_{s}", name=f"h0_{s}")
                nc.scalar.activation(out=h0[:], in_=ph0[:], func=SIG)
                h0T.append(h0)

            # ---- layer-1 heads + final projection -------------------------
            for s in range(2):
                mh = fp.tile([P, NB], F32, tag="mh")
                mhv = mh[:].rearrange("p (b h) -> p b h", h=2)
                view = h1T[:, NS1 * s:NS1 * (s + 1)].rearrange(
                    "p (b h j) -> p b h j", h=2, j=25)
                nc.vector.tensor_reduce(out=mhv, in_=view,
                                        axis=mybir.AxisListType.X,
                                        op=mybir.AluOpType.add)
                for k in range(3):
                    pz = pm.tile([P, BL], F32, tag="ph", space="PSUM")
                    nc.tensor.matmul(out=pz[:], lhsT=wh[k][0][:],
                                     rhs=h0T[s][:], start=True, stop=False)
                    nc.tensor.matmul(out=pz[:], lhsT=wh[k][1][:],
                                     rhs=mhv[:, :, 0], start=False, stop=False)
                    nc.tensor.matmul(out=pz[:], lhsT=wh[k][2][:],
                                     rhs=mhv[:, :, 1], start=False, stop=True)
                    zt = fp.tile([P, BL], F32, tag="zt")
                    nc.scalar.activation(out=zt[:], in_=pz[:], func=SIG)
                    po = pm.tile([D, BL], F32, tag="po", space="PSUM")
                    nc.tensor.matmul(out=po[:], lhsT=wd[k][:], rhs=zt[:],
                                     start=True, stop=True)
                    ot = fp.tile([D, BL], F32, tag="ot")
                    nc.vector.tensor_copy(out=ot[:], in_=po[:])
                    nc.sync.dma_start(out=out_d[s * 3 + k, :, :], in_=ot[:])

    nc.compile()
    return nc


_NC_CACHE = None


def _get_nc():
    global _NC_CACHE
    if _NC_CACHE is None:
        _NC_CACHE = build_program()
    return _NC_CACHE


def _core_indices(nodes1c, nodes2c, nb_out, nb_in):
    """Host-side replica of the reference sampling for one core.
    Returns (nloc [64] i32, s1c [128, 25] i32, ids2 [128, 500] i32)."""
    nloc = np.concatenate([nodes1c, nodes2c]).astype(np.int64)  # [64]
    s1 = np.empty((NB, 50), dtype=np.int64)
    s1[:BL] = np.concatenate([nb_out[nloc[:BL]][:, S1_C1_OUT],
                              nb_in[nloc[:BL]][:, S1_C1_IN]], axis=1)
    s1[BL:] = np.concatenate([nb_out[nloc[BL:]][:, S2_C1_OUT],
                              nb_in[nloc[BL:]][:, S2_C1_IN]], axis=1)
    s1f = s1.reshape(-1)  # sample i = base*50 + j; tile slot: i = t*128 + p
    ids2 = np.empty((NS, 20), dtype=np.int64)
    ids2[:NS1] = np.concatenate([nb_out[s1f[:NS1]][:, S1_C2_OUT],
                                 nb_in[s1f[:NS1]][:, S1_C2_IN]], axis=1)
    ids2[NS1:] = np.concatenate([nb_out[s1f[NS1:]][:, S2_C2_OUT],
                                 nb_in[s1f[NS1:]][:, S2_C2_IN]], axis=1)
    s1c = np.ascontiguousarray(
        s1f.reshape(NT, P).T.astype(np.int32))              # [128, 25]
    ids2h = np.ascontiguousarray(
        ids2.reshape(NT, P, 20).transpose(1, 0, 2)
        .reshape(P, NT * 20).astype(np.int32))              # [128, 500]
    return nloc.astype(np.int32), s1c, ids2h


def host_prep(nodes1, nodes2, neighbors_out, neighbors_in, features,
              W_in, W_mean, W_std, W_pi, Wd_mean, Wd_std, Wd_pi):
    nodes1 = np.asarray(nodes1, dtype=np.int64)
    nodes2 = np.asarray(nodes2, dtype=np.int64)
    nb_out = np.asarray(neighbors_out, dtype=np.int64)
    nb_in = np.asarray(neighbors_in, dtype=np.int64)
    features = np.ascontiguousarray(np.asarray(features, dtype=np.float32))

    def scale(w, f):
        w = np.array(w, dtype=np.float32, copy=True)
        w[F:] *= np.float32(1.0 / f)
        return w

    w1 = scale(W_in, 10.0)
    w0 = scale(W_in, 25.0)
    whs = [scale(W_mean, 25.0), scale(W_std, 25.0), scale(W_pi, 25.0)]
    wds = [np.ascontiguousarray(np.asarray(w, dtype=np.float32))
           for w in (Wd_mean, Wd_std, Wd_pi)]

    in_maps = []
    for c in range(NCORES):
        nloc, s1c, ids2h = _core_indices(nodes1[c * BL:(c + 1) * BL],
                                         nodes2[c * BL:(c + 1) * BL],
                                         nb_out, nb_in)
        m = {"nodes": nloc, "feats": features, "s1c": s1c, "ids2": ids2h,
             "w1": w1, "w0": w0}
        for k in range(3):
            m[f"wh{k}"] = whs[k]
            m[f"wd{k}"] = wds[k]
        in_maps.append(m)
    return in_maps


def kernel(nodes1, nodes2, neighbors_out, neighbors_in, features,
           W_in, W_mean, W_std, W_pi, W_ag, W_ad, Wd_mean, Wd_std, Wd_pi,
           _trace=False):
    in_maps = host_prep(nodes1, nodes2, neighbors_out, neighbors_in, features,
                        W_in, W_mean, W_std, W_pi, Wd_mean, Wd_std, Wd_pi)
    nc = _get_nc()
    from concourse.bass_utils import run_bass_kernel_spmd
    res = run_bass_kernel_spmd(nc, in_maps, list(range(NCORES)),
                               trace=_trace)
    if _trace:
        kernel.last_results = res

    out = np.zeros((6, B, D), dtype=np.float32)
    for c in range(NCORES):
        o = res.results[c]["out"]  # [6, D, BL]
        for i in range(6):
            out[i, c * BL:(c + 1) * BL, :] = o[i].T
    return out
